# revision 1
# baseline (speedup 1.0000x reference)
"""AttentionBlock kernel for 8 Trainium2 NeuronCores (Bass/Tile).

Problem (hardcoded shapes): x [16, 512, 32, 32] fp32, GroupNorm(32 groups,
eps=1e-5) -> 1x1-conv QKV (qkv_w [1536,512], qkv_b) -> 8-head attention over
T=1024 positions (head dim 64) -> 1x1-conv proj -> residual add.

Sharding: pure data-parallel over batch; each of the 8 cores handles 2
batches end-to-end; weights replicated; no collectives.

Per-core dataflow (per batch, all layouts channel-on-partition [128, ko, T]):
  1. GroupNorm stats per channel via bn_stats/bn_aggr, group reduction via a
     tiny constant matmul (G: [512,32] one-hot/16), broadcast back to
     channels via a second constant matmul (B = G^T one-hot), then
     tensor_scalar normalize.  norm_w/norm_b are folded into the QKV weights
     host-side, the 1/sqrt(64) attention scale and the q bias are folded into
     Wq/bq, the k bias is dropped (softmax shift invariance), and the v bias
     is folded into the proj bias.
  2. q,k = Wqk @ h as [128, T] head-pairs (head h occupies partitions
     64*(h%2)..); v^T computed directly as h^T @ Wv^T (no transposes needed).
  3. Per head: St = kz^T q in [s, t] layout (kz zero-padded to K=128 — PE
     tiling-mode switches corrupt in-flight matmuls on this HW, so every
     matmul stays in 128-row mode), exp on ScalarE (psum->sbuf, bf16),
     AV+denominator in one matmul with lhsT = [v^T | ones] (denominator
     lands replicated on the opposite 64 partitions), 1/D via magic-seed +
     2 Newton iterations (standard ops on GPSIMD/DVE — the custom-DVE
     reciprocal misfires on this HW), one sbuf->sbuf DMA to lane-shift the
     reciprocal onto the numerator partitions, one tensor_tensor multiply.
     Attention is software-pipelined: head h's St/exp stream interleaves
     with head h-1's AV matmuls to keep the PE dense.
  4. proj matmul + (residual + proj bias) add, DMA out.
"""

import numpy as np

B, C, T = 16, 512, 1024
NH, CH = 8, 64
NG = 32
EPS = 1e-5
NCORES = 8
BPC = B // NCORES  # batches per core
KO = C // 128      # channel chunks

# --- dtype configuration -------------------------------------------------
# 'f32'  : plain float32 matmuls (4 cycles/row on PE)
# 'f32r' : float32 data, matmul operands bitcast to float32r (1 cycle/row)
# 'bf16' : operands stored/cast to bfloat16 (1 cycle/row)
MM_QKV = 'bf16'   # h, wqkT, wvT operand treatment (qkv + v^T matmuls)
MM_ATT = 'bf16'   # q, k, expSt, vT operand treatment (St + AV matmuls)
MM_PROJ = 'bf16'  # a, wpT operand treatment (proj matmul)
TRACE = False
DEBUG = False  # adds intermediate-tensor outputs (h, q, k, vt, es(head0), a)
DEBUG_LIGHT = False  # only h + a outputs (minimal schedule perturbation)
DEBUG_QKV = False  # dump q/kz/vt at end of batch (no attention perturbation)
DEBUG_ATT = False  # dump r(h0) after TT and es(h0) after pair-0 AV
ATT_BLOCKED = True  # St for a head-pair blocked before AVs (fewer PE
                    # tiling-mode switches); needs 16 live expSt tiles.


def _npdt(mode):
    import ml_dtypes
    return np.dtype(ml_dtypes.bfloat16) if mode == 'bf16' else np.float32


def _build_nc():
    import concourse.bass as bass
    import concourse.tile as tile
    from concourse import bacc, mybir
    from contextlib import ExitStack

    f32 = mybir.dt.float32
    f32r = mybir.dt.float32r
    bf16 = mybir.dt.bfloat16

    def mmdt(mode):
        # float32r tiles: same bytes as fp32, but the producing op rounds on
        # write and the PE runs the matmul at full (1 cycle/row) rate.
        return {'bf16': bf16, 'f32r': f32r, 'f32': f32}[mode]

    def mm_ap(ap, mode):
        return ap

    dt_h = mmdt(MM_QKV)    # h tile dtype (rhs of qkv, lhsT of v^T)
    dt_att = mmdt(MM_ATT)  # q, k, expSt, vT tiles
    dt_a = mmdt(MM_PROJ)   # a tile

    # Bacc (not raw Bass): its finalize() runs move_matmul_waits_to_ldweights
    # + generate_event_semaphores, which split multi-sem waits into the 1-wait
    # form walrus codegen requires.
    nc = bacc.Bacc()
    AF = mybir.ActivationFunctionType
    ALU = mybir.AluOpType

    x_d = nc.dram_tensor("x", [BPC, 128, KO, T], f32, kind="ExternalInput")
    wqk_d = nc.dram_tensor("wqkT", [128, KO, 2 * C], mmdt(MM_QKV), kind="ExternalInput")
    wv_d = nc.dram_tensor("wvT", [128, KO, C], mmdt(MM_QKV), kind="ExternalInput")
    wp_d = nc.dram_tensor("wpT", [128, KO, C], mmdt(MM_PROJ), kind="ExternalInput")
    bq_d = nc.dram_tensor("bq", [128, KO], f32, kind="ExternalInput")
    bp_d = nc.dram_tensor("bp", [128, KO], f32, kind="ExternalInput")
    g_d = nc.dram_tensor("gmat", [128, KO, NG], f32, kind="ExternalInput")
    b_d = nc.dram_tensor("bmat", [128, KO, 128], f32, kind="ExternalInput")
    ones_d = nc.dram_tensor("ones", [128, 64], mmdt(MM_ATT), kind="ExternalInput")
    out_d = nc.dram_tensor("out", [BPC, 128, KO, T], f32, kind="ExternalOutput")
    if DEBUG_LIGHT:
        dbg_h = nc.dram_tensor("dbg_h", [BPC, 128, KO, T], dt_h, kind="ExternalOutput")
        dbg_a = nc.dram_tensor("dbg_a", [BPC, 128, KO, T], dt_a, kind="ExternalOutput")
    if DEBUG_QKV:
        dbg_q = nc.dram_tensor("dbg_q", [BPC, 128, KO, T], dt_att, kind="ExternalOutput")
        dbg_kz = nc.dram_tensor("dbg_kz", [BPC, 128, NH, T], dt_att, kind="ExternalOutput")
        dbg_vt = nc.dram_tensor("dbg_vt", [BPC, 128, 8, 4, 192], dt_att, kind="ExternalOutput")
    if DEBUG_ATT:
        dbg_r = nc.dram_tensor("dbg_r", [BPC, 128, T], f32, kind="ExternalOutput")
        dbg_d = nc.dram_tensor("dbg_d", [BPC, 128, T], f32, kind="ExternalOutput")
        dbg_es = nc.dram_tensor("dbg_es", [BPC, 8, 128, T], dt_att, kind="ExternalOutput")
    if DEBUG:
        dbg_h = nc.dram_tensor("dbg_h", [BPC, 128, KO, T], dt_h, kind="ExternalOutput")
        dbg_q = nc.dram_tensor("dbg_q", [BPC, 128, KO, T], dt_att, kind="ExternalOutput")
        dbg_k = nc.dram_tensor("dbg_k", [BPC, 128, KO, T], dt_att, kind="ExternalOutput")
        dbg_vt = nc.dram_tensor("dbg_vt", [BPC, 128, 8, 4, 192], dt_att, kind="ExternalOutput")
        dbg_es = nc.dram_tensor("dbg_es", [BPC, 8, 128, T], dt_att, kind="ExternalOutput")
        dbg_a = nc.dram_tensor("dbg_a", [BPC, 128, KO, T], dt_a, kind="ExternalOutput")

    # Every matmul keeps the PE in the default 128-row tiling mode (operands
    # zero-padded to K=128 where needed).  Switching the array tiling mode
    # (e.g. K=64 row-tiling) without a drain corrupts in-flight matmuls on
    # HW, and nothing in this stack inserts that drain — so we never switch.
    def mm(out, lhsT, rhs, **kw):
        assert lhsT.partition_size() == 128
        return nc.tensor.matmul(out, lhsT, rhs, **kw)

    with tile.TileContext(nc) as tc, ExitStack() as ctx:
        consts = ctx.enter_context(tc.tile_pool(name="consts", bufs=1))
        xp = ctx.enter_context(tc.tile_pool(name="xp", bufs=2))
        hp = ctx.enter_context(tc.tile_pool(name="hp", bufs=1))
        qkp = ctx.enter_context(tc.tile_pool(name="qkp", bufs=1))
        vtp = ctx.enter_context(tc.tile_pool(name="vtp", bufs=1))
        esp = ctx.enter_context(tc.tile_pool(name="esp", bufs=24))
        rp = ctx.enter_context(tc.tile_pool(name="rp", bufs=2))
        ap_ = ctx.enter_context(tc.tile_pool(name="ap", bufs=2))
        gnp = ctx.enter_context(tc.tile_pool(name="gnp", bufs=2))
        psS = ctx.enter_context(tc.tile_pool(name="psS", bufs=2, space="PSUM"))
        psB = ctx.enter_context(tc.tile_pool(name="psB", bufs=2, space="PSUM"))

        # constants
        wqk_sb = consts.tile([128, KO, 2 * C], mmdt(MM_QKV))
        nc.sync.dma_start(wqk_sb[:], wqk_d[:])
        wv_sb = consts.tile([128, KO, C], mmdt(MM_QKV))
        nc.sync.dma_start(wv_sb[:], wv_d[:])
        wp_sb = consts.tile([128, KO, C], mmdt(MM_PROJ))
        nc.sync.dma_start(wp_sb[:], wp_d[:])
        bq_sb = consts.tile([128, KO], f32)
        nc.sync.dma_start(bq_sb[:], bq_d[:])
        bp_sb = consts.tile([128, KO], f32)
        nc.sync.dma_start(bp_sb[:], bp_d[:])
        g_sb = consts.tile([128, KO, NG], f32)
        nc.sync.dma_start(g_sb[:], g_d[:])
        bm_sb = consts.tile([128, KO, 128], f32)
        nc.sync.dma_start(bm_sb[:], b_d[:])

        # v^T lhsT buffer: per head-pair p the 192 columns are
        # [vT_even(64) | ones(64) | vT_odd(64)]; head 2p uses cols 0:128 of
        # the block ([vT|ones]) and head 2p+1 uses cols 64:192 ([ones|vT]).
        eps_sb = consts.tile([NG, 1], f32)
        nc.vector.memset(eps_sb[:], EPS)

        # Magic seed constant for the Newton reciprocal (fast-inverse trick:
        # y0_bits = 0x7EF127EA - x_bits, ~5% seed error).
        i32 = mybir.dt.int32
        magic_sb = consts.tile([128, 2], i32)
        nc.vector.memset(magic_sb[:], 0x7EF127EA)

        for b in range(BPC):
            x_sb = xp.tile([128, KO, T], f32, tag="x")
            nc.sync.dma_start(x_sb[:], x_d[b])

            # v^T lhsT buffer (fresh per batch; ones blocks re-DMA'd so every
            # consumer's ordering is within-batch)
            vt_sb = vtp.tile([128, 8, 4, 192], dt_att, tag="vt")
            ones_src = bass.AP(tensor=ones_d, offset=0,
                               ap=[[64, 128], [0, 32], [1, 64]])
            vt_flat = vt_sb[:].rearrange("p a b w -> p (a b) w")
            nc.sync.dma_start(vt_flat[:, :, 64:128], ones_src)

            # ---------------- GroupNorm ----------------
            rhs3 = gnp.tile([128, KO, 3], f32, tag="rhs3")
            for ko in range(KO):
                stats = gnp.tile([128, 2, 6], f32, tag="stats")
                for j in range(2):
                    nc.vector.bn_stats(out=stats[:, j, :], in_=x_sb[:, ko, 512 * j:512 * (j + 1)])
                nc.vector.bn_aggr(out=rhs3[:, ko, 0:2], in_=stats[:])
                nc.vector.tensor_mul(rhs3[:, ko, 2:3], rhs3[:, ko, 0:1], rhs3[:, ko, 0:1])
            gps = psS.tile([NG, 3], f32, tag="st")
            for ko in range(KO):
                mm(gps[:], g_sb[:, ko, :], rhs3[:, ko, :],
                                 start=(ko == 0), stop=(ko == KO - 1))
            # var = E[var] + E[mean^2] - mean^2 ; rstd = 1/sqrt(var+eps)
            gq = gnp.tile([NG, 3], f32, tag="gq")
            nc.vector.tensor_copy(gq[:], gps[:])
            gtmp = gnp.tile([NG, 2], f32, tag="gtmp")
            # [128, 2]: rows 32..127 zeroed — they meet the zero-padded rows
            # of bmat in the broadcast matmul (rhs K must be 128).
            gst2 = gnp.tile([128, 2], f32, tag="gst2")
            nc.vector.memset(gst2[:], 0.0)
            nc.vector.tensor_copy(gst2[0:NG, 0:1], gq[:, 0:1])
            nc.vector.tensor_add(gtmp[:, 0:1], gq[:, 1:2], gq[:, 2:3])
            nc.vector.tensor_mul(gtmp[:, 1:2], gq[:, 0:1], gq[:, 0:1])
            nc.vector.tensor_sub(gtmp[:, 0:1], gtmp[:, 0:1], gtmp[:, 1:2])
            nc.scalar.activation(gtmp[:, 1:2], gtmp[:, 0:1], AF.Sqrt, bias=eps_sb[:])
            nc.vector.reciprocal(gst2[0:NG, 1:2], gtmp[:, 1:2])
            bst_ps = psS.tile([128, 2 * KO], f32, tag="st")
            for ko in range(KO):
                mm(bst_ps[:, 2 * ko:2 * ko + 2], bm_sb[:, ko, :], gst2[:],
                                 start=True, stop=True)
            bst = gnp.tile([128, 2 * KO], f32, tag="bst_sb")
            nc.vector.tensor_copy(bst[:], bst_ps[:])
            h_sb = hp.tile([128, KO, T], dt_h, tag="h")
            for ko in range(KO):
                nc.vector.tensor_scalar(
                    out=h_sb[:, ko, :], in0=x_sb[:, ko, :],
                    scalar1=bst[:, 2 * ko:2 * ko + 1], scalar2=bst[:, 2 * ko + 1:2 * ko + 2],
                    op0=ALU.subtract, op1=ALU.mult)
            # pre-add proj bias to residual x (x := x + bp per channel)
            for ko in range(KO):
                nc.vector.tensor_scalar(
                    out=x_sb[:, ko, :], in0=x_sb[:, ko, :],
                    scalar1=bp_sb[:, ko:ko + 1], scalar2=None, op0=ALU.add)

            # ---------------- QKV (q,k) ----------------
            # kz: one zero-padded [128, T] lhsT tile per head — head h's k on
            # partitions 64*(h%2)..+64, zeros elsewhere, so St runs at K=128
            # against the shared q pair tile with no PE tiling-mode switch.
            q_sb = qkp.tile([128, KO, T], dt_att, tag="q")
            kz_sb = qkp.tile([128, NH, T], dt_att, tag="kz")
            nc.vector.memset(kz_sb[64:128, 0:NH:2, :], 0.0)
            nc.vector.memset(kz_sb[0:64, 1:NH:2, :], 0.0)
            # k chunks interleaved first so head 0's St can start early
            for m in (4, 0, 5, 1, 6, 2, 7, 3):
                for half in range(2):
                    pq = psS.tile([128, 512], f32, tag="st")
                    for ko in range(KO):
                        mm(
                            pq[:], mm_ap(wqk_sb[:, ko, 128 * m:128 * (m + 1)], MM_QKV),
                            mm_ap(h_sb[:, ko, 512 * half:512 * (half + 1)], MM_QKV),
                            start=(ko == 0), stop=(ko == KO - 1))
                    if m < 4:
                        nc.vector.tensor_scalar(
                            out=q_sb[:, m, 512 * half:512 * (half + 1)], in0=pq[:],
                            scalar1=bq_sb[:, m:m + 1], scalar2=None, op0=ALU.add)
                    else:
                        p = m - 4
                        sl = slice(512 * half, 512 * (half + 1))
                        nc.vector.tensor_copy(kz_sb[0:64, 2 * p, sl], pq[0:64, :])
                        nc.vector.tensor_copy(kz_sb[64:128, 2 * p + 1, sl], pq[64:128, :])

            # ---------------- v^T ----------------
            for tc_i in range(8):
                pv = psS.tile([128, 512], f32, tag="st")
                for ko in range(KO):
                    mm(
                        pv[:], mm_ap(h_sb[:, ko, 128 * tc_i:128 * (tc_i + 1)], MM_QKV),
                        mm_ap(wv_sb[:, ko, :], MM_QKV),
                        start=(ko == 0), stop=(ko == KO - 1))
                pvv = pv[:].rearrange("p (h c) -> p h c", c=CH)
                nc.vector.tensor_copy(vt_sb[:, tc_i, :, 0:64], pvv[:, 0:NH:2, :])
                nc.vector.tensor_copy(vt_sb[:, tc_i, :, 128:192], pvv[:, 1:NH:2, :])

            # ---------------- attention ----------------
            def emit_st(h):
                """St = k^T q for head h -> list of 8 expSt sbuf tiles.

                lhsT = kz (head's k zero-padded to 128 partitions), rhs = the
                shared q pair tile; the other head's q rows meet zero weights.
                """
                p = h // 2
                es_tiles = []
                for sc in range(8):
                    es = esp.tile([128, T], dt_att, tag="es")
                    for half in range(2):
                        st = psS.tile([128, 512], f32, tag="st")
                        mm(
                            st[:],
                            mm_ap(kz_sb[:, h, 128 * sc:128 * (sc + 1)], MM_ATT),
                            mm_ap(q_sb[:, p, 512 * half:512 * (half + 1)], MM_ATT),
                            start=True, stop=True)
                        nc.scalar.activation(
                            es[:, 512 * half:512 * (half + 1)], st[:], AF.Exp)
                    es_tiles.append(es)
                return es_tiles

            def av_mms(avp, h_av, es_av, sc):
                p, e = h_av // 2, h_av % 2
                es = es_av[sc]
                for half in range(2):
                    mm(
                        avp[half][:], mm_ap(vt_sb[:, sc, p, 64 * e:64 * e + 128], MM_ATT),
                        mm_ap(es[:, 512 * half:512 * (half + 1)], MM_ATT),
                        start=(sc == 0), stop=(sc == 7))

            def finish_norm(h_av, av, a_sb, fast=False):
                # fast=True: run the Newton ops on DVE (lower latency) — used
                # for the batch's last head, whose chain gates proj while the
                # PE sits idle.
                p, e = h_av // 2, h_av % 2
                b0, b1 = 64 * e, 64 * (1 - e)
                # r = 1/D via magic-seed + 2 Newton iterations on GPSIMD.
                # (The custom-DVE reciprocal_approx_fast computes garbage
                # nondeterministically on this HW/ucode build even with a
                # correct, in-order staged input — so it is not used.  The
                # Newton chain is standard ops only; DVE stages the PSUM read,
                # GPSIMD computes, keeping DVE off the critical path.)
                r = rp.tile([128, T], f32, tag="r")
                dsb = rp.tile([128, 2 * T], f32, tag="dsb")
                for half in range(2):
                    nc.vector.tensor_copy(
                        dsb[b1:b1 + 64, 512 * half:512 * (half + 1)],
                        av[half][b1:b1 + 64, :])
                dD = dsb[b1:b1 + 64, 0:T]             # D (positive)
                tt = dsb[b1:b1 + 64, T:2 * T]         # scratch (same lanes)
                ry = r[b1:b1 + 64, :]
                i32 = mybir.dt.int32
                eng = nc.vector if fast else nc.gpsimd
                eng.tensor_tensor(                     # y0 = bits(magic - D_bits)
                    out=ry.bitcast(i32), in0=magic_sb[b1:b1 + 64, 0:1].to_broadcast((64, T)),
                    in1=dD.bitcast(i32), op=ALU.subtract)
                eng.tensor_mul(tt, dD, ry)             # t = D*y0
                nc.vector.scalar_tensor_tensor(        # z1 = (t-2)*y0 = -y1
                    out=ry, in0=tt, scalar=2.0, in1=ry,
                    op0=ALU.subtract, op1=ALU.mult)
                eng.tensor_mul(tt, dD, ry)             # t2 = D*z1 (negative)
                nc.vector.scalar_tensor_tensor(        # z2 = (t2+2)*z1 = -y2
                    out=ry, in0=tt, scalar=2.0, in1=ry,
                    op0=ALU.add, op1=ALU.mult)
                nc.vector.tensor_scalar_mul(ry, ry, -1.0)  # r = y2 ~ 1/D
                nc.sync.dma_start(out=r[b0:b0 + 64, :], in_=r[b1:b1 + 64, :])
                for half in range(2):
                    nc.vector.tensor_tensor(
                        out=a_sb[b0:b0 + 64, p, 512 * half:512 * (half + 1)],
                        in0=av[half][b0:b0 + 64, :],
                        in1=r[b0:b0 + 64, 512 * half:512 * (half + 1)], op=ALU.mult)

            # Software-pipelined attention: head h's St matmuls (PE, gated by
            # the trailing ScalarE exp stream) are interleaved with head h-1's
            # AV matmuls (PE, inputs long ready) — keeps the PE dense and warm
            # while ACT catches up, all in the one K=128 tiling mode.
            a_sb = ap_.tile([128, KO, T], dt_a, tag="a")
            prev = None  # (head, es_tiles)
            for h in range(NH):
                avp = None
                if prev is not None:
                    avp = [psB.tile([128, 512], f32, tag=f"av{i}", name=f"av{i}")
                           for i in range(2)]
                p = h // 2
                es_tiles = []
                for sc in range(8):
                    es = esp.tile([128, T], dt_att, tag="es")
                    st = psS.tile([128, T], f32, tag="st")
                    for half in range(2):
                        mm(
                            st[:, 512 * half:512 * (half + 1)],
                            mm_ap(kz_sb[:, h, 128 * sc:128 * (sc + 1)], MM_ATT),
                            mm_ap(q_sb[:, p, 512 * half:512 * (half + 1)], MM_ATT),
                            start=True, stop=True)
                    nc.scalar.activation(es[:], st[:], AF.Exp)
                    if avp is not None:
                        av_mms(avp, prev[0], prev[1], sc)
                    es_tiles.append(es)
                if avp is not None:
                    finish_norm(prev[0], avp, a_sb)
                prev = (h, es_tiles)
            avp = [psB.tile([128, 512], f32, tag=f"av{i}", name=f"av{i}")
                   for i in range(2)]
            for sc in range(8):
                av_mms(avp, prev[0], prev[1], sc)
            finish_norm(prev[0], avp, a_sb)
            if DEBUG_LIGHT:
                nc.sync.dma_start(dbg_h[b], h_sb[:])
                nc.sync.dma_start(dbg_a[b], a_sb[:])
            if DEBUG_QKV:
                nc.sync.dma_start(dbg_q[b], q_sb[:])
                nc.sync.dma_start(dbg_kz[b], kz_sb[:])
                nc.sync.dma_start(dbg_vt[b], vt_sb[:])
            if DEBUG:
                nc.sync.dma_start(dbg_h[b], h_sb[:])
                nc.sync.dma_start(dbg_q[b], q_sb[:])
                nc.sync.dma_start(dbg_k[b], k_sb[:])
                nc.sync.dma_start(dbg_vt[b], vt_sb[:])
                nc.sync.dma_start(dbg_a[b], a_sb[:])

            # ---------------- proj + residual ----------------
            # Residual add lands in-place in x_sb (this op is x's last
            # reader), so no separate output buffer: the h pool slot frees at
            # vT time and batch b+1's normalize/qkv can overlap this batch's
            # tail instead of waiting for the output DMA.
            for m in range(KO):
                for half in range(2):
                    po = psS.tile([128, 512], f32, tag="st")
                    for ko in range(KO):
                        mm(
                            po[:], mm_ap(wp_sb[:, ko, 128 * m:128 * (m + 1)], MM_PROJ),
                            mm_ap(a_sb[:, ko, 512 * half:512 * (half + 1)], MM_PROJ),
                            start=(ko == 0), stop=(ko == KO - 1))
                    nc.vector.tensor_add(
                        x_sb[:, m, 512 * half:512 * (half + 1)], po[:],
                        x_sb[:, m, 512 * half:512 * (half + 1)])
            nc.sync.dma_start(out_d[b], x_sb[:])

    if not nc.is_finalized():
        nc.finalize()
    return nc


def _prep_inputs(x, norm_w, norm_b, qkv_w, qkv_b, proj_w, proj_b):
    """Fold norms/biases/scale into weights; reshape for the kernel layout."""
    f = np.float32
    x = np.asarray(x, f)
    nw = np.asarray(norm_w, f)
    nb = np.asarray(norm_b, f)
    qkv_w = np.asarray(qkv_w, f)
    qkv_b = np.asarray(qkv_b, f)
    proj_w = np.asarray(proj_w, f)
    proj_b = np.asarray(proj_b, f)

    Wq, Wk, Wv = qkv_w[0:C], qkv_w[C:2 * C], qkv_w[2 * C:3 * C]
    bqv, bkv, bvv = qkv_b[0:C], qkv_b[C:2 * C], qkv_b[2 * C:3 * C]
    scale = f(1.0 / np.sqrt(CH))
    Wq_e = (Wq * nw[None, :]) * scale
    bq_e = (Wq @ nb + bqv) * scale
    Wk_e = Wk * nw[None, :]          # k bias dropped (softmax shift invariance)
    Wv_e = Wv * nw[None, :]
    bv_e = Wv @ nb + bvv
    bp_e = proj_b + proj_w @ bv_e    # v bias folded into proj bias

    def chan_chunks(vec):  # [C] -> [128, KO]
        return np.ascontiguousarray(vec.reshape(KO, 128).T)

    def lhsT_chunks(wT, dtype):  # [C, M] -> [128, KO, M]
        return np.ascontiguousarray(
            wT.reshape(KO, 128, wT.shape[1]).transpose(1, 0, 2)).astype(dtype)

    wqkT = np.concatenate([Wq_e, Wk_e], axis=0).T  # [C, 1024]
    gm = np.zeros((C, NG), f)
    gm[np.arange(C), np.arange(C) // (C // NG)] = 1.0 / (C // NG)
    # bm zero-padded to 128 rows so the broadcast matmul runs at K=128
    bm = np.zeros((128, C), f)
    bm[np.arange(C) // (C // NG), np.arange(C)] = 1.0

    dqkv = _npdt(MM_QKV)
    dproj = _npdt(MM_PROJ)
    shared = {
        "wqkT": lhsT_chunks(wqkT, dqkv),
        "wvT": lhsT_chunks(Wv_e.T, dqkv),
        "wpT": lhsT_chunks(proj_w.T, dproj),
        "bq": chan_chunks(bq_e),
        "bp": chan_chunks(bp_e),
        "gmat": np.ascontiguousarray(
            gm.reshape(KO, 128, NG).transpose(1, 0, 2)),
        "bmat": np.ascontiguousarray(bm.reshape(128, KO, 128)),
        "ones": np.ones((128, 64), _npdt(MM_ATT)),
    }
    xr = x.reshape(B, C, T)
    in_maps = []
    for c in range(NCORES):
        xc = xr[c * BPC:(c + 1) * BPC].reshape(BPC, KO, 128, T).transpose(0, 2, 1, 3)
        m = dict(shared)
        m["x"] = np.ascontiguousarray(xc)
        in_maps.append(m)
    return in_maps


LDW_OPT = False  # walrus --enable-ldw-opt=true fails codegen (visitInstLdweights)


def _patch_ldw_opt():
    import concourse.bass_utils as bu

    if getattr(bu, "_ldw_opt_patched", False):
        return
    orig = bu.run_command

    def patched(argv, **kw):
        argv = ["--enable-ldw-opt=true" if a == "--enable-ldw-opt=false" else a
                for a in argv]
        return orig(argv, **kw)

    bu.run_command = patched
    bu._ldw_opt_patched = True


def kernel(x, norm_w, norm_b, qkv_w, qkv_b, proj_w, proj_b):
    from concourse.bass_utils import run_bass_kernel_spmd

    if LDW_OPT:
        _patch_ldw_opt()

    in_maps = _prep_inputs(x, norm_w, norm_b, qkv_w, qkv_b, proj_w, proj_b)
    nc = _build_nc()
    res = run_bass_kernel_spmd(nc, in_maps, core_ids=list(range(NCORES)), trace=TRACE)
    kernel.last_results = res
    outs = []
    for c in range(NCORES):
        oc = res.results[c]["out"]  # [BPC, 128, KO, T]
        outs.append(np.asarray(oc).transpose(0, 2, 1, 3).reshape(BPC, C, T))
    full = np.concatenate(outs, axis=0).reshape(B, C, 32, 32).astype(np.float32)
    return full

